# revision 23
# baseline (speedup 1.0000x reference)
"""Trainium2 Bass kernel for nn_MultiHeadAttention_63591285785308.

Reference semantics (faithful "reshape, no transpose" head split):
  Q = query @ Wq.T + bq            [B, S, D]
  Qh = Q.reshape(B, H, S, dk)       # head h <-> rows h*128:(h+1)*128 of Q[b]
  scores = Qh @ Kh^T / sqrt(dk); P = softmax(scores); ctx = P @ Vh
  out = ctx.reshape(B, S, D) @ Wo.T + bo

Because the head split is a flat reshape, unit (b, h) depends only on the
128-row slab query[b, h*128:(h+1)*128] (same for key/value), and writes only
out[b, h*128:(h+1)*128].  32 units are sharded 4-per-core across 8 cores.

On-device layout: all activations are kept feature-on-partition ("T"
orientation).  Head-sequence positions use the j-major permutation
s' = j*128 + r (the true position is s = 16*r + j) which makes every
PSUM->SBUF copy and gather a contiguous [64, 128] block; attention is
permutation-invariant along s and t, so only the host-visible input/output
mapping (which works in r) needs no correction at all.

Compute dtype bf16 (fp32 accumulation in PSUM); softmax denominator via an
extra ones-column in the PV matmul, divided out in fp32.
"""

import json

import numpy as np
import ml_dtypes

B, S, D, H, DK, P = 2, 2048, 1024, 16, 64, 128
NCORES = 8
UPC = 4  # units per core

_BF16 = ml_dtypes.bfloat16
_prog_cache = {}

_MAX_SYNC = 2  # this walrus build allows at most 2 sync commands per instruction


def _legalize_bir_sync(bir_bytes):
    """Split sync waits/updates exceeding the per-instruction cap onto
    adjacent same-engine NoOps (engine program order preserves semantics:
    waits move to preceding nops, update overflow to trailing nops)."""
    d = json.loads(bir_bytes)
    ctr = [0]

    def nop(engine, debug, waits, upds):
        ctr[0] += 1
        return {
            "debug": debug,
            "engine": engine,
            "ins": [],
            "name": f"I-lgl{ctr[0]}",
            "opcode": "NoOp",
            "outs": [],
            "sync_info": {"on_update": upds, "on_wait": waits},
        }

    changed = False
    for fn in d["functions"]:
        for blk in fn["blocks"]:
            new = []
            for ins in blk["instructions"]:
                si = ins.get("sync_info") or {}
                waits = list(si.get("on_wait") or [])
                upds = list(si.get("on_update") or [])
                if len(waits) + len(upds) <= _MAX_SYNC:
                    new.append(ins)
                    continue
                changed = True
                keep_u = upds[:_MAX_SYNC]
                extra_u = upds[_MAX_SYNC:]
                keep_w = waits[: max(0, _MAX_SYNC - len(keep_u))]
                extra_w = waits[len(keep_w):]
                # NoOp lowers to a CTRL-type op whose sync budget is 1 on
                # some engines (Pool) — put exactly one wait/update per nop.
                for w in extra_w:
                    new.append(nop(ins["engine"], ins.get("debug", 0), [w], []))
                si["on_wait"] = keep_w
                si["on_update"] = keep_u
                ins["sync_info"] = si
                new.append(ins)
                for uu in extra_u:
                    new.append(nop(ins["engine"], ins.get("debug", 0), [], [uu]))
            blk["instructions"] = new
    if not changed:
        return bir_bytes
    return json.dumps(d).encode()


def _install_bir_legalizer():
    if _prog_cache.get("legalizer_installed"):
        return
    from concourse import bass2jax

    orig = bass2jax.compile_bir_kernel

    def patched(ant_bir_str, compile_dir, neff_name="file.neff", **kw):
        return orig(_legalize_bir_sync(ant_bir_str), compile_dir, neff_name=neff_name, **kw)

    bass2jax.compile_bir_kernel = patched
    _prog_cache["legalizer_installed"] = True


def _build_program():
    import concourse.bass as bass
    import concourse.mybir as mybir
    import concourse.tile as tile
    from concourse.vector_clock import ScopedClock, VectorClock
    from concourse.masks import make_identity

    dt = mybir.dt
    BF = dt.bfloat16
    F32 = dt.float32
    F32R = dt.float32r
    ADD = mybir.AluOpType.add
    MUL = mybir.AluOpType.mult
    EXP = mybir.ActivationFunctionType.Exp

    class SplitDrainTileContext(tile.TileContext):
        """This walrus build caps sem waits per instruction below what the
        stock tail drain needs; split the waits across single-wait SP nops
        (SP program order then gates the bare drain)."""

        def _drain_and_barrier(self, tick_clock, wait_clock):
            gc = tick_clock.global_clock
            for proc in range(len(gc)):
                tick = gc[proc]
                if tick <= 0:
                    continue
                vc = VectorClock()
                vc.require_at_least(proc, tick)
                nop = self.nc.sync.nop(nofuse=True)
                wait_clock.add_sem_waits(nop.ins, ScopedClock({None: vc}))
            self.nc.sync.drain()
            self.nc.all_engine_barrier()
            assert self.sems is not None
            popped = self.nc._tile_sem_poison_stack.pop()
            assert popped is self._sem_poison
            self.nc.clear_and_free_semaphores(list(self.sems.allocated().values()))
            self.nc.all_engine_barrier()

    nc = bass.Bass()

    xq_d = nc.declare_dram_parameter("xqT", [D, 512], BF, isOutput=False)
    xk_d = nc.declare_dram_parameter("xkT", [D, 512], BF, isOutput=False)
    xv_d = nc.declare_dram_parameter("xvT", [D, 512], BF, isOutput=False)
    wq_d = nc.declare_dram_parameter("wqT", [D, D], BF, isOutput=False)
    wk_d = nc.declare_dram_parameter("wkT", [D, D], BF, isOutput=False)
    wv_d = nc.declare_dram_parameter("wvT", [D, D], BF, isOutput=False)
    wo_d = nc.declare_dram_parameter("woT", [D, D], BF, isOutput=False)
    bq_d = nc.declare_dram_parameter("bq", [P, 8], F32, isOutput=False)
    bk_d = nc.declare_dram_parameter("bk", [P, 8], F32, isOutput=False)
    bv_d = nc.declare_dram_parameter("bv", [P, 8], F32, isOutput=False)
    bo_d = nc.declare_dram_parameter("bo", [1, D], BF, isOutput=False)
    out_d = nc.declare_dram_parameter("out", [UPC, P, D], F32, isOutput=True)

    with SplitDrainTileContext(nc) as tc:
        with (
            tc.tile_pool(name="persist", bufs=1) as pp,
            tc.tile_pool(name="pt", bufs=3) as ptpool,
            tc.tile_pool(name="gather", bufs=3) as gpool,
            tc.tile_pool(name="cun", bufs=2) as cupool,
            tc.tile_pool(name="ostage", bufs=2) as opool,
            tc.tile_pool(name="den", bufs=2) as dpool,
            tc.tile_pool(name="wide_ps", bufs=2, space="PSUM") as wps,
            tc.tile_pool(name="narrow_ps", bufs=4, space="PSUM") as nps,
        ):
            # Load order matters: Q-proj needs wq+xq first; wo only at the
            # very end.  Two tiles per tensor (chunks 0-3 / 4-7) balance
            # dependency granularity against per-DMA issue cost (~0.7us).
            w_sb = {}
            x_sb = {}
            for nm in ("q", "k", "v", "o"):
                w_sb[nm] = [pp.tile([P, 4, D], BF, name=f"w_{nm}{i}", tag=f"w_{nm}{i}")
                            for i in range(2)]
            for nm in ("q", "k", "v"):
                x_sb[nm] = [pp.tile([P, 4, 512], BF, name=f"x_{nm}{i}", tag=f"x_{nm}{i}")
                            for i in range(2)]
            for nm, wd, xd in (("q", wq_d, xq_d), ("k", wk_d, xk_d), ("v", wv_d, xv_d)):
                wr = wd.rearrange("(c p) o -> p c o", p=P)
                xr = xd.rearrange("(c p) s -> p c s", p=P)
                for i in range(2):
                    nc.sync.dma_start(out=w_sb[nm][i][:], in_=wr[:, i * 4:(i + 1) * 4, :])
                    nc.sync.dma_start(out=x_sb[nm][i][:], in_=xr[:, i * 4:(i + 1) * 4, :])
            wo_r = wo_d.rearrange("(c p) o -> p c o", p=P)
            for i in range(2):
                nc.sync.dma_start(out=w_sb["o"][i][:], in_=wo_r[:, i * 4:(i + 1) * 4, :])
            b_sb = {}
            for nm, d in (("q", bq_d), ("k", bk_d), ("v", bv_d)):
                t = pp.tile([P, 8], F32, name=f"b_{nm}", tag=f"b_{nm}")
                nc.sync.dma_start(out=t[:], in_=d[:])
                b_sb[nm] = t
            bo_sb = pp.tile([1, D], BF, name="bo", tag="bo")
            nc.sync.dma_start(out=bo_sb[:], in_=bo_d[:])

            ident = pp.tile([P, P], BF, name="ident", tag="ident")
            make_identity(nc, ident)
            ones_bf = pp.tile([1, P], BF, name="ones_bf", tag="ones_bf")
            nc.gpsimd.memset(ones_bf, 1.0)
            ones_rf = pp.tile([1, 64], F32, name="ones_rf", tag="ones_rf")
            nc.gpsimd.memset(ones_rf, 1.0)
            ones_r = pp.tile([1, 64], F32R, name="ones_r", tag="ones_r")
            nc.vector.tensor_copy(out=ones_r[:], in_=ones_rf[:])

            # head-T buffers, pair-packed: unit pr*2+pu on partitions pu*64..
            qhT = [pp.tile([P, S], BF, name=f"qhT{pr}", tag=f"qhT{pr}") for pr in range(2)]
            khT = [pp.tile([P, S], BF, name=f"khT{pr}", tag=f"khT{pr}") for pr in range(2)]
            vhT = [pp.tile([P, S], BF, name=f"vhT{pr}", tag=f"vhT{pr}") for pr in range(2)]
            ctxT = [pp.tile([P, S], BF, name=f"ctxT{pr}", tag=f"ctxT{pr}") for pr in range(2)]
            # Vh natural chunks per unit, with a trailing ones column
            # (strided non-zero memset is invalid ISA here, so memset a
            # contiguous ones tile and copy it into the strided column)
            ones_col = pp.tile([P, 16], BF, name="ones_col", tag="ones_col")
            nc.gpsimd.memset(ones_col, 1.0)
            vh = [pp.tile([P, 16, 65], BF, name=f"vh{u}", tag=f"vh{u}") for u in range(UPC)]
            for u in range(UPC):
                nc.vector.tensor_copy(out=vh[u][:, :, 64], in_=ones_col[:])

            dst_of = {"q": qhT, "k": khT, "v": vhT}

            for pr in range(2):
                # ---- projections (both units of the pair batched, N=256)
                for nm in ("q", "k", "v"):
                    for c in range(8):
                        ps = nps.tile([P, 512], F32, tag="narrow", name=f"proj_{pr}_{nm}_{c}")
                        pjp = ps[:, 0:256]
                        for i in range(8):
                            nc.tensor.matmul(
                                pjp,
                                lhsT=w_sb[nm][i // 4][:, i % 4, c * P:(c + 1) * P],
                                rhs=x_sb[nm][i // 4][:, i % 4, pr * 256:(pr + 1) * 256],
                                start=(i == 0),
                                stop=(i == 7),
                            )
                        for pu in range(2):
                            for hh in range(2):
                                j = 2 * c + hh
                                dst = dst_of[nm][pr][pu * 64:pu * 64 + 64, j * P:(j + 1) * P]
                                src = pjp[hh * 64:(hh + 1) * 64, pu * P:(pu + 1) * P]
                                bias = b_sb[nm][hh * 64:(hh + 1) * 64, c:c + 1].to_broadcast((64, P))
                                nc.vector.tensor_tensor(out=dst, in0=src, in1=bias, op=ADD)

                # ---- Vh transposes (serves both units of the pair at once)
                for cc in range(16):
                    tp = nps.tile([P, P], BF, tag="narrow", name=f"tp_{pr}_{cc}")
                    nc.tensor.transpose(tp[:, 0:P], vhT[pr][:, cc * P:(cc + 1) * P], ident)
                    for pu in range(2):
                        u = pr * 2 + pu
                        nc.vector.tensor_copy(out=vh[u][:, cc, 0:64], in_=tp[:, pu * 64:pu * 64 + 64])

                # ---- attention + output projection per unit
                for pu in range(2):
                    u = pr * 2 + pu
                    prow = slice(pu * 64, pu * 64 + 64)
                    for sb in range(2):
                        ctxps = [
                            nps.tile([P, 512], F32, tag="narrow", name=f"ctx_{u}_{sb}_{q}")
                            for q in range(2)
                        ]
                        for tt in range(16):
                            sc = wps.tile([P, 1024], F32, tag="wide", name=f"sc_{u}_{sb}_{tt}")
                            for q in range(2):
                                s0 = sb * 1024 + q * 512
                                nc.tensor.matmul(
                                    sc[:, q * 512:(q + 1) * 512],
                                    lhsT=khT[pr][prow, tt * P:(tt + 1) * P],
                                    rhs=qhT[pr][prow, s0:s0 + 512],
                                    start=True,
                                    stop=True,
                                )
                            pt = ptpool.tile([P, 1024], BF, tag="pt", name=f"pt_{u}_{sb}_{tt}")
                            nc.scalar.activation(pt[:], sc[:], EXP, scale=0.125)
                            for q in range(2):
                                nc.tensor.matmul(
                                    ctxps[q][0:65, :],
                                    lhsT=vh[u][:, tt, :],
                                    rhs=pt[:, q * 512:(q + 1) * 512],
                                    start=(tt == 0),
                                    stop=(tt == 15),
                                    skip_group_check=True,
                                )
                        cun = cupool.tile([65, 1024], F32, tag="cun", name=f"cun_{u}_{sb}")
                        for q in range(2):
                            nc.vector.tensor_copy(out=cun[:, q * 512:(q + 1) * 512], in_=ctxps[q][0:65, :])
                        for q in range(2):
                            s0 = sb * 1024 + q * 512
                            rec = dpool.tile([1, 512], F32, tag="rec", name=f"rec_{u}_{sb}_{q}")
                            nc.vector.reciprocal(rec[:], cun[64:65, q * 512:(q + 1) * 512])
                            rec_r = dpool.tile([1, 512], F32R, tag="den", name=f"den_{u}_{sb}_{q}")
                            nc.vector.tensor_copy(out=rec_r[:], in_=rec[:])
                            bc = nps.tile([P, 512], F32, tag="narrow", name=f"bc_{u}_{sb}_{q}")
                            nc.tensor.matmul(
                                bc[0:64, :],
                                lhsT=ones_r[0:1, :],
                                rhs=rec_r[:],
                                start=True,
                                stop=True,
                            )
                            nc.vector.tensor_tensor(
                                out=ctxT[pr][prow, s0:s0 + 512],
                                in0=cun[0:64, q * 512:(q + 1) * 512],
                                in1=bc[0:64, :],
                                op=MUL,
                            )

                    # ---- output projection for unit u
                    ops = [
                        nps.tile([P, 512], F32, tag="narrow", name=f"out_{u}_{ot}")
                        for ot in range(2)
                    ]
                    for c in range(8):
                        g = gpool.tile([P, P], BF, tag="gather", name=f"g_{u}_{c}")
                        for hh in range(2):
                            j = 2 * c + hh
                            nc.vector.tensor_copy(
                                out=g[hh * 64:(hh + 1) * 64, :],
                                in_=ctxT[pr][prow, j * P:(j + 1) * P],
                            )
                        for ot in range(2):
                            nc.tensor.matmul(
                                ops[ot],
                                lhsT=g[:],
                                rhs=w_sb["o"][c // 4][:, c % 4, ot * 512:(ot + 1) * 512],
                                start=(c == 0),
                                stop=False,
                                skip_group_check=True,
                            )
                    for ot in range(2):
                        nc.tensor.matmul(
                            ops[ot],
                            lhsT=ones_bf[:, :],
                            rhs=bo_sb[:, ot * 512:(ot + 1) * 512],
                            start=False,
                            stop=True,
                            skip_group_check=True,
                        )
                        ostg = opool.tile([P, 512], F32, tag="ostage", name=f"ostg_{u}_{ot}")
                        nc.vector.tensor_copy(out=ostg[:], in_=ops[ot])
                        nc.sync.dma_start(out=out_d[u, :, ot * 512:(ot + 1) * 512], in_=ostg[:])

    return nc


def _get_program():
    if "nc" not in _prog_cache:
        _prog_cache["nc"] = _build_program()
    return _prog_cache["nc"]


def _prepare_in_maps(query, key, value, Wq, bq, Wk, bk, Wv, bv, Wo, bo):
    wqT = np.ascontiguousarray(Wq.T).astype(_BF16)
    wkT = np.ascontiguousarray(Wk.T).astype(_BF16)
    wvT = np.ascontiguousarray(Wv.T).astype(_BF16)
    woT = np.ascontiguousarray(Wo.T).astype(_BF16)
    bq2 = np.ascontiguousarray(bq.reshape(8, P).T).astype(np.float32)
    bk2 = np.ascontiguousarray(bk.reshape(8, P).T).astype(np.float32)
    bv2 = np.ascontiguousarray(bv.reshape(8, P).T).astype(np.float32)
    bo2 = bo.reshape(1, D).astype(_BF16)

    in_maps = []
    for core in range(NCORES):
        units = [core * UPC + k for k in range(UPC)]
        slabs = {}
        for nm, full in (("xqT", query), ("xkT", key), ("xvT", value)):
            cols = [
                np.ascontiguousarray(full[u // H, (u % H) * P:(u % H + 1) * P, :].T)
                for u in units
            ]
            slabs[nm] = np.concatenate(cols, axis=1).astype(_BF16)
        in_maps.append(
            {
                **slabs,
                "wqT": wqT, "wkT": wkT, "wvT": wvT, "woT": woT,
                "bq": bq2, "bk": bk2, "bv": bv2, "bo": bo2,
            }
        )
    return in_maps


def kernel(query, key, value, Wq, bq, Wk, bk, Wv, bv, Wo, bo, _trace=False):
    from concourse.bass_utils import run_bass_kernel_spmd

    _install_bir_legalizer()

    query = np.asarray(query, dtype=np.float32)
    key = np.asarray(key, dtype=np.float32)
    value = np.asarray(value, dtype=np.float32)

    nc = _get_program()
    in_maps = _prepare_in_maps(query, key, value,
                               np.asarray(Wq), np.asarray(bq), np.asarray(Wk),
                               np.asarray(bk), np.asarray(Wv), np.asarray(bv),
                               np.asarray(Wo), np.asarray(bo))
    core_ids = list(range(NCORES))
    res = run_bass_kernel_spmd(nc, in_maps, core_ids, trace=_trace)
    _prog_cache["last_results"] = res

    out = np.empty((B, S, D), np.float32)
    for core in range(NCORES):
        o = res.results[core]["out"]
        for k in range(UPC):
            u = core * UPC + k
            out[u // H, (u % H) * P:(u % H + 1) * P, :] = o[k]
    return out
